# revision 13
# baseline (speedup 1.0000x reference)
"""Trainium2 Bass kernel for capsule dynamic routing (nn_Capsule).

Reference computation:
    hat = (x @ kernel).reshape(B, N, C, D).transpose(0, 2, 1, 3)   # [B,C,N,D]
    b = 0; 3 routing iterations of:
        w = softmax(b, axis=capsules)
        o = squash(einsum('bcn,bcnd->bcd', w, hat))
        b += einsum('bcd,bcnd->bcn', o, hat)

Key reformulation (hat is never materialized):
    o[c,d]  = sum_i xw[c,i] * K[i,(c,d)]      xw = w[c,:] @ x      (A-op)
    bupd[c,n] = sum_i x[n,i] * oK[c,i]        oK[c,i] = sum_d o[c,d]*K[i,(c,d)]
This reduces 34 GFLOP of hat-work to ~100 MFLOP of small matmuls whose cost
is streaming K through the PE as stationary operands (bf16, fp32 accumulate).

Sharding: data-parallel over batch B=16 across 8 cores (2 items/core, fused
into the same matmuls via a 2-wide moving operand). kernel K replicated.
"""

import numpy as np
import ml_dtypes
from contextlib import ExitStack

NCORES = 8
B, N, DI = 16, 512, 256         # batch, input capsules, input dim
C, D = 64, 128                  # output capsules, capsule dim
NB = B // NCORES                # batch items per core
P = 128                         # SBUF partitions
NCH = N // P                    # 4 n-chunks
ICH = DI // P                   # 2 i-chunks
ROUTINGS = 3
EPS = 1e-7

_cache = {}


def _build_program(reps=0):
    """reps=0: plain single-shot program (graded path).
    reps>0: wrap the whole body (input DMA + compute + output DMA) in a
    For_i loop for wall-clock-difference benchmarking."""
    import concourse.mybir as mybir
    import concourse.tile as tile
    from concourse import bacc
    from concourse.masks import make_identity

    F32 = mybir.dt.float32
    BF16 = mybir.dt.bfloat16
    AF = mybir.ActivationFunctionType
    AX = mybir.AxisListType

    nc = bacc.Bacc("TRN2", target_bir_lowering=False, debug=False)

    x_d = nc.dram_tensor("x", [P, NB, NCH, DI], BF16, kind="ExternalInput").ap()
    xT_d = nc.dram_tensor("xT", [P, NB, ICH, N], BF16, kind="ExternalInput").ap()
    kk_d = nc.dram_tensor("kk", [P, ICH, C * D], BF16, kind="ExternalInput").ap()
    kt_d = nc.dram_tensor("kt", [P, C, DI], BF16, kind="ExternalInput").ap()
    xbar_d = nc.dram_tensor("xbar", [P, ICH, NB], BF16, kind="ExternalInput").ap()
    out_d = nc.dram_tensor("out", [P, D], F32, kind="ExternalOutput").ap()

    with tile.TileContext(nc) as tc, ExitStack() as ctx:
        big = ctx.enter_context(tc.tile_pool(name="big", bufs=1))
        wk = ctx.enter_context(tc.tile_pool(name="wk", bufs=2))
        psA = ctx.enter_context(tc.tile_pool(name="psA", bufs=2, space="PSUM"))
        psT = ctx.enter_context(tc.tile_pool(name="psT", bufs=1, space="PSUM"))
        psk = ctx.enter_context(tc.tile_pool(name="psk", bufs=1, space="PSUM"))
        pss = ctx.enter_context(tc.tile_pool(name="pss", bufs=3, space="PSUM"))

        kk = big.tile([P, ICH, C * D], BF16)
        kt = big.tile([P, C, DI], BF16)
        xs = big.tile([P, NB, NCH, DI], BF16)
        xT = big.tile([P, NB, ICH, N], BF16)
        xbar = big.tile([P, ICH, NB], BF16)

        def emit_input_dmas():
            # kk is needed first (A-op of iteration 0) — chunk it so capsule
            # matmuls can start while later chunks are still in flight.
            KCH = 8
            kw = (C * D) // KCH
            for j in range(KCH):
                nc.sync.dma_start(
                    out=kk[:, :, j * kw:(j + 1) * kw],
                    in_=kk_d[:, :, j * kw:(j + 1) * kw],
                )
            nc.sync.dma_start(out=xbar, in_=xbar_d)
            nc.sync.dma_start(out=xs, in_=x_d)
            nc.sync.dma_start(out=xT, in_=xT_d)
            for j in range(KCH):
                cs = C // KCH
                nc.sync.dma_start(
                    out=kt[:, j * cs:(j + 1) * cs, :],
                    in_=kt_d[:, j * cs:(j + 1) * cs, :],
                )

        idf = big.tile([P, P], F32)
        make_identity(nc, idf)
        idb = big.tile([P, P], BF16)
        make_identity(nc, idb)

        bT = big.tile([P, NB, NCH, C], F32)     # routing logits, [n, b, nch, c]
        o_bf = big.tile([P, C, NB], BF16)       # current o, [d, (c,b)]
        epst = big.tile([P, 1], F32)
        nc.vector.memset(epst, EPS)
        zerot = big.tile([P, 1], F32)
        nc.vector.memset(zerot, 0.0)

        def a_op(rhs_at):
            """o_pre[d,(c,b)] = sum_i rhs[i,(c,b)] * K[i,(c,d)] per capsule."""
            po = psA.tile([P, C, NB], F32, tag="po")
            for c in range(C):
                for t in range(ICH):
                    nc.tensor.matmul(
                        po[:, c, :],
                        lhsT=kk[:, t, c * D:(c + 1) * D],
                        rhs=rhs_at(t, c),
                        start=(t == 0),
                        stop=(t == ICH - 1),
                    )
            return po

        def squash(po, last):
            opre = wk.tile([P, C * NB], F32, tag="opre")
            nc.vector.tensor_copy(out=opre, in_=po[:].rearrange("p c b -> p (c b)"))
            pot = psT.tile([P, P], F32, tag="pot")   # [(c,b), d]
            nc.tensor.transpose(pot, opre, idf)
            sqs = wk.tile([P, D], F32, tag="sqs")
            ssum = wk.tile([P, 1], F32, tag="ssum")
            nc.scalar.activation(out=sqs, in_=pot, func=AF.Square, accum_out=ssum)
            scale = wk.tile([P, 1], F32, tag="scale")
            nc.scalar.activation(out=scale, in_=ssum, func=AF.Sqrt, bias=epst[:])
            den = wk.tile([P, 1], F32, tag="den")
            nc.vector.tensor_scalar_add(den, ssum, 0.5 + EPS)
            nc.vector.reciprocal(den, den)
            nc.vector.tensor_mul(scale, scale, den)  # sqrt(s+eps)/(0.5+s+eps)
            if last:
                oout = wk.tile([P, D], F32, tag="oout")
                nc.vector.tensor_scalar_mul(oout, pot, scale)
                nc.sync.dma_start(out=out_d, in_=oout)
            else:
                osc = wk.tile([P, D], BF16, tag="osc")
                nc.vector.tensor_scalar_mul(osc, pot, scale)
                pob = psT.tile([P, P], BF16, tag="pob")
                nc.tensor.transpose(pob, osc, idb)   # back to [d, (c,b)]
                nc.vector.tensor_copy(
                    out=o_bf[:].rearrange("p c b -> p (c b)"), in_=pob
                )

        def b_op(it):
            # oK[i,(c,b)] per i-tile: contraction over d on partitions.
            pk = psk.tile([P, ICH, C, NB], F32, tag="pk")
            for c in range(C):
                for t in range(ICH):
                    nc.tensor.matmul(
                        pk[:, t, c, :],
                        lhsT=kt[:, c, t * P:(t + 1) * P],
                        rhs=o_bf[:, c, :],
                        start=True,
                        stop=True,
                    )
            oks = wk.tile([P, NB, ICH, C], BF16, tag="oks")
            for b in range(NB):
                nc.vector.tensor_copy(out=oks[:, b], in_=pk[:, :, :, b])
            # bupd[n,(c)] = sum_i x[n,i] oK[c,i]  (lhsT = xT tiles)
            for b in range(NB):
                for nt in range(NCH):
                    pb = pss.tile([P, C], F32, tag="pb")
                    for t in range(ICH):
                        nc.tensor.matmul(
                            pb,
                            lhsT=xT[:, b, t, nt * P:(nt + 1) * P],
                            rhs=oks[:, b, t, :],
                            start=(t == 0),
                            stop=(t == ICH - 1),
                        )
                    if it == 0:
                        nc.vector.tensor_copy(out=bT[:, b, nt], in_=pb)
                    else:
                        nc.vector.tensor_add(out=bT[:, b, nt], in0=bT[:, b, nt], in1=pb)

        def softmax_xw():
            # softmax over capsules (innermost free axis of bT); values are
            # O(1) so the max-subtraction is unnecessary.
            e = wk.tile([P, NB, NCH, C], F32, tag="e")
            nc.scalar.activation(out=e, in_=bT[:], func=AF.Exp, bias=zerot[:])
            es = wk.tile([P, NB, NCH], F32, tag="es")
            nc.vector.reduce_sum(out=es, in_=e, axis=AX.X)
            nc.vector.reciprocal(es, es)
            w = wk.tile([P, NB, NCH, C], BF16, tag="w")
            nc.vector.tensor_tensor(
                w, e, es[:, :, :, None].to_broadcast((P, NB, NCH, C)),
                mybir.AluOpType.mult,
            )
            # xwT[i,(c,b)] = sum_n x[n,i] w[n,c]  (lhsT = x tiles)
            xwT = wk.tile([P, ICH, C, NB], BF16, tag="xwT")
            for b in range(NB):
                for t in range(ICH):
                    px = pss.tile([P, C], F32, tag="pb")
                    for ch in range(NCH):
                        nc.tensor.matmul(
                            px,
                            lhsT=xs[:, b, ch, t * P:(t + 1) * P],
                            rhs=w[:, b, ch, :],
                            start=(ch == 0),
                            stop=(ch == NCH - 1),
                        )
                    nc.vector.tensor_copy(out=xwT[:, t, :, b], in_=px)
            return xwT

        def body():
            emit_input_dmas()
            po = a_op(lambda t, c: xbar[:, t, :])
            squash(po, last=False)
            for it in range(ROUTINGS - 1):
                b_op(it)
                xwT = softmax_xw()
                po = a_op(lambda t, c, _x=xwT: _x[:, t, c, :])
                squash(po, last=(it == ROUTINGS - 2))

        if reps:
            with tc.For_i(0, reps, 1, hint_engines=(mybir.EngineType.PE,)):
                body()
        else:
            body()

    nc.compile()
    return nc


def _prep_inputs(x, kernel):
    bf16 = ml_dtypes.bfloat16
    kk = np.ascontiguousarray(
        kernel.reshape(ICH, P, C * D).transpose(1, 0, 2)).astype(bf16)
    kt = np.ascontiguousarray(
        kernel.reshape(DI, C, D).transpose(2, 1, 0)).astype(bf16)
    in_maps = []
    for s in range(NCORES):
        xc = x[s * NB:(s + 1) * NB]                      # [NB, N, DI]
        x_in = np.ascontiguousarray(
            xc.reshape(NB, NCH, P, DI).transpose(2, 0, 1, 3)).astype(bf16)
        xT_in = np.ascontiguousarray(
            xc.reshape(NB, N, ICH, P).transpose(3, 0, 2, 1)).astype(bf16)
        xb = xc.sum(axis=1) / C                          # [NB, DI] fp32
        xbar_in = np.ascontiguousarray(
            xb.reshape(NB, ICH, P).transpose(2, 1, 0)).astype(bf16)
        in_maps.append(
            {"x": x_in, "xT": xT_in, "kk": kk, "kt": kt, "xbar": xbar_in}
        )
    return in_maps


def kernel(x, kernel, _trace=False, _reps=0):
    from concourse.bass_utils import run_bass_kernel_spmd

    x = np.ascontiguousarray(np.asarray(x, dtype=np.float32))
    kernel = np.ascontiguousarray(np.asarray(kernel, dtype=np.float32))
    assert x.shape == (B, N, DI) and kernel.shape == (DI, C * D)

    key = ("nc", _reps)
    if key not in _cache:
        _cache[key] = _build_program(reps=_reps)
    nc = _cache[key]

    in_maps = _prep_inputs(x, kernel)
    res = run_bass_kernel_spmd(nc, in_maps, list(range(NCORES)), trace=_trace)
    _cache["last_result"] = res

    out = np.empty((B, C, D), dtype=np.float32)
    for s in range(NCORES):
        o = res.results[s]["out"]                        # [(c,b), d]
        out[s * NB:(s + 1) * NB] = o.reshape(C, NB, D).transpose(1, 0, 2)
    return out


# revision 16
# speedup vs baseline: 1.1721x; 1.1721x over previous
"""Trainium2 Bass kernel for capsule dynamic routing (nn_Capsule).

Reference computation:
    hat = (x @ kernel).reshape(B, N, C, D).transpose(0, 2, 1, 3)   # [B,C,N,D]
    b = 0; 3 routing iterations of:
        w = softmax(b, axis=capsules)
        o = squash(einsum('bcn,bcnd->bcd', w, hat))
        b += einsum('bcd,bcnd->bcn', o, hat)

Key reformulation (hat is never materialized):
    o[c,d]  = sum_i xw[c,i] * K[i,(c,d)]      xw = w[c,:] @ x      (A-op)
    bupd[c,n] = sum_i x[n,i] * oK[c,i]        oK[c,i] = sum_d o[c,d]*K[i,(c,d)]
This reduces 34 GFLOP of hat-work to ~100 MFLOP of small matmuls whose cost
is streaming K through the PE as stationary operands (bf16, fp32 accumulate).

Sharding: data-parallel over batch B=16 across 8 cores (2 items/core, fused
into the same matmuls via a 2-wide moving operand). kernel K replicated.
"""

import numpy as np
import ml_dtypes
from contextlib import ExitStack

NCORES = 8
B, N, DI = 16, 512, 256         # batch, input capsules, input dim
C, D = 64, 128                  # output capsules, capsule dim
NB = B // NCORES                # batch items per core
P = 128                         # SBUF partitions
NCH = N // P                    # 4 n-chunks
ICH = DI // P                   # 2 i-chunks
ROUTINGS = 3
EPS = 1e-7

_cache = {}


def _build_program(reps=0):
    """reps=0: plain single-shot program (graded path).
    reps>0: wrap the whole body (input DMA + compute + output DMA) in a
    For_i loop for wall-clock-difference benchmarking."""
    import concourse.mybir as mybir
    import concourse.tile as tile
    from concourse import bacc
    from concourse.masks import make_identity

    F32 = mybir.dt.float32
    BF16 = mybir.dt.bfloat16
    AF = mybir.ActivationFunctionType
    AX = mybir.AxisListType

    class _OneActSetBacc(bacc.Bacc):
        """Every activation func used here (Square/Ln/Exp/Copy) lives in the
        'natural_log_exp_and_others' table set, but the default chooser picks
        per-func sets greedily and flip-flops (one ~1.3us LoadActFuncSet per
        switch, on the critical path). Mask the other sets so exactly one
        table load is emitted; indices are preserved so act_func_set_id still
        points at the real act_info.json entry."""

        def insert_act_table_loads(self):
            import bass_rust as _br
            from concourse.hw_specs import get_activation_tables

            has_activation = any(
                isinstance(i, mybir.InstActivation)
                for b in self.main_func.blocks
                for i in b.instructions
            )
            if not has_activation:
                return
            tables = [
                (name, funcs if name == "natural_log_exp_and_others" else set())
                for name, funcs in get_activation_tables(self.m.arch).items()
            ]
            _br.insert_act_table_loads(self, tables)

    nc = _OneActSetBacc("TRN2", target_bir_lowering=False, debug=False)

    x_d = nc.dram_tensor("x", [P, NB, NCH, DI], BF16, kind="ExternalInput").ap()
    xT_d = nc.dram_tensor("xT", [P, NB, ICH, N], BF16, kind="ExternalInput").ap()
    kk_d = nc.dram_tensor("kk", [P, ICH, C * D], BF16, kind="ExternalInput").ap()
    kt_d = nc.dram_tensor("kt", [P, C, DI], BF16, kind="ExternalInput").ap()
    xbar_d = nc.dram_tensor("xbar", [P, ICH, NB], BF16, kind="ExternalInput").ap()
    out_d = nc.dram_tensor("out", [P, D], F32, kind="ExternalOutput").ap()

    with tile.TileContext(nc) as tc, ExitStack() as ctx:
        big = ctx.enter_context(tc.tile_pool(name="big", bufs=1))
        wk = ctx.enter_context(tc.tile_pool(name="wk", bufs=2))
        psA = ctx.enter_context(tc.tile_pool(name="psA", bufs=2, space="PSUM"))
        psT = ctx.enter_context(tc.tile_pool(name="psT", bufs=1, space="PSUM"))
        psk = ctx.enter_context(tc.tile_pool(name="psk", bufs=1, space="PSUM"))
        pss = ctx.enter_context(tc.tile_pool(name="pss", bufs=3, space="PSUM"))

        kk = big.tile([P, ICH, C * D], BF16)
        kt = big.tile([P, C, DI], BF16)
        xs = big.tile([P, NB, NCH, DI], BF16)
        xT = big.tile([P, NB, ICH, N], BF16)
        xbar = big.tile([P, ICH, NB], BF16)

        def emit_input_dmas():
            # DMA bandwidth is shared; order by first use: xbar + kk feed the
            # iteration-0 A-op, kt feeds b_op, xT feeds bupd, xs feeds xwT.
            # kk/kt are chunked so capsule matmuls start while later chunks
            # are still in flight.
            nc.sync.dma_start(out=xbar, in_=xbar_d)
            KCH = 8
            kw = (C * D) // KCH
            for j in range(KCH):
                nc.sync.dma_start(
                    out=kk[:, :, j * kw:(j + 1) * kw],
                    in_=kk_d[:, :, j * kw:(j + 1) * kw],
                )
            for j in range(KCH):
                cs = C // KCH
                nc.sync.dma_start(
                    out=kt[:, j * cs:(j + 1) * cs, :],
                    in_=kt_d[:, j * cs:(j + 1) * cs, :],
                )
            nc.sync.dma_start(out=xT, in_=xT_d)
            nc.sync.dma_start(out=xs, in_=x_d)

        idf = big.tile([P, P], F32)
        make_identity(nc, idf)
        idb = big.tile([P, P], BF16)
        make_identity(nc, idb)

        bT = big.tile([P, NB, NCH, C], F32)     # routing logits, [n, b, nch, c]
        o_bf = big.tile([P, C, NB], BF16)       # current o, [d, (c,b)]
        epst = big.tile([P, 1], F32)
        nc.vector.memset(epst, EPS)
        zerot = big.tile([P, 1], F32)
        nc.vector.memset(zerot, 0.0)

        def a_op(rhs_at):
            """o_pre[d,(c,b)] = sum_i rhs[i,(c,b)] * K[i,(c,d)] per capsule."""
            po = psA.tile([P, C, NB], F32, tag="po")
            for c in range(C):
                for t in range(ICH):
                    nc.tensor.matmul(
                        po[:, c, :],
                        lhsT=kk[:, t, c * D:(c + 1) * D],
                        rhs=rhs_at(t, c),
                        start=(t == 0),
                        stop=(t == ICH - 1),
                    )
            return po

        def squash(po, last):
            opre = wk.tile([P, C * NB], F32, tag="opre")
            nc.vector.tensor_copy(out=opre, in_=po[:].rearrange("p c b -> p (c b)"))
            pot = psT.tile([P, P], F32, tag="pot")   # [(c,b), d]
            nc.tensor.transpose(pot, opre, idf)
            sqs = wk.tile([P, D], F32, tag="sqs")
            ssum = wk.tile([P, 1], F32, tag="ssum")
            nc.scalar.activation(out=sqs, in_=pot, func=AF.Square, accum_out=ssum)
            # sqrt(s+eps) = exp(0.5*ln(s+eps)): Ln/Exp/Square live in the same
            # activation-table set, so no LoadActFuncSet (~1.3us) per squash.
            scale = wk.tile([P, 1], F32, tag="scale")
            nc.scalar.activation(out=scale, in_=ssum, func=AF.Ln, bias=epst[:])
            nc.scalar.activation(out=scale, in_=scale, func=AF.Exp, scale=0.5,
                                 bias=zerot[:])
            den = wk.tile([P, 1], F32, tag="den")
            nc.vector.tensor_scalar_add(den, ssum, 0.5 + EPS)
            nc.vector.reciprocal(den, den)
            nc.vector.tensor_mul(scale, scale, den)  # sqrt(s+eps)/(0.5+s+eps)
            if last:
                oout = wk.tile([P, D], F32, tag="oout")
                nc.vector.tensor_scalar_mul(oout, pot, scale)
                nc.sync.dma_start(out=out_d, in_=oout)
            else:
                osc = wk.tile([P, D], BF16, tag="osc")
                nc.vector.tensor_scalar_mul(osc, pot, scale)
                pob = psT.tile([P, P], BF16, tag="pob")
                nc.tensor.transpose(pob, osc, idb)   # back to [d, (c,b)]
                nc.vector.tensor_copy(
                    out=o_bf[:].rearrange("p c b -> p (c b)"), in_=pob
                )

        def b_op(it):
            # oK[i,(c,b)] per i-tile: contraction over d on partitions.
            pk = psk.tile([P, ICH, C, NB], F32, tag="pk")
            for c in range(C):
                for t in range(ICH):
                    nc.tensor.matmul(
                        pk[:, t, c, :],
                        lhsT=kt[:, c, t * P:(t + 1) * P],
                        rhs=o_bf[:, c, :],
                        start=True,
                        stop=True,
                    )
            oks = wk.tile([P, NB, ICH, C], BF16, tag="oks")
            for b in range(NB):
                nc.vector.tensor_copy(out=oks[:, b], in_=pk[:, :, :, b])
            # bupd[n,(c)] = sum_i x[n,i] oK[c,i]  (lhsT = xT tiles)
            for b in range(NB):
                for nt in range(NCH):
                    pb = pss.tile([P, C], F32, tag="pb")
                    for t in range(ICH):
                        nc.tensor.matmul(
                            pb,
                            lhsT=xT[:, b, t, nt * P:(nt + 1) * P],
                            rhs=oks[:, b, t, :],
                            start=(t == 0),
                            stop=(t == ICH - 1),
                        )
                    if it == 0:
                        nc.vector.tensor_copy(out=bT[:, b, nt], in_=pb)
                    else:
                        nc.vector.tensor_add(out=bT[:, b, nt], in0=bT[:, b, nt], in1=pb)

        def softmax_xw():
            # softmax over capsules (innermost free axis of bT); values are
            # O(1) so the max-subtraction is unnecessary.
            e = wk.tile([P, NB, NCH, C], F32, tag="e")
            nc.scalar.activation(out=e, in_=bT[:], func=AF.Exp, bias=zerot[:])
            es = wk.tile([P, NB, NCH], F32, tag="es")
            nc.vector.reduce_sum(out=es, in_=e, axis=AX.X)
            nc.vector.reciprocal(es, es)
            w = wk.tile([P, NB, NCH, C], BF16, tag="w")
            nc.vector.tensor_tensor(
                w, e, es[:, :, :, None].to_broadcast((P, NB, NCH, C)),
                mybir.AluOpType.mult,
            )
            # xwT[i,(c,b)] = sum_n x[n,i] w[n,c]  (lhsT = x tiles)
            xwT = wk.tile([P, ICH, C, NB], BF16, tag="xwT")
            for b in range(NB):
                for t in range(ICH):
                    px = pss.tile([P, C], F32, tag="pb")
                    for ch in range(NCH):
                        nc.tensor.matmul(
                            px,
                            lhsT=xs[:, b, ch, t * P:(t + 1) * P],
                            rhs=w[:, b, ch, :],
                            start=(ch == 0),
                            stop=(ch == NCH - 1),
                        )
                    nc.vector.tensor_copy(out=xwT[:, t, :, b], in_=px)
            return xwT

        def body():
            emit_input_dmas()
            po = a_op(lambda t, c: xbar[:, t, :])
            squash(po, last=False)
            for it in range(ROUTINGS - 1):
                b_op(it)
                xwT = softmax_xw()
                po = a_op(lambda t, c, _x=xwT: _x[:, t, c, :])
                squash(po, last=(it == ROUTINGS - 2))

        if reps:
            with tc.For_i(0, reps, 1, hint_engines=(mybir.EngineType.PE,)):
                body()
        else:
            body()

    nc.compile()
    return nc


def _prep_inputs(x, kernel):
    bf16 = ml_dtypes.bfloat16
    kk = np.ascontiguousarray(
        kernel.reshape(ICH, P, C * D).transpose(1, 0, 2)).astype(bf16)
    kt = np.ascontiguousarray(
        kernel.reshape(DI, C, D).transpose(2, 1, 0)).astype(bf16)
    in_maps = []
    for s in range(NCORES):
        xc = x[s * NB:(s + 1) * NB]                      # [NB, N, DI]
        x_in = np.ascontiguousarray(
            xc.reshape(NB, NCH, P, DI).transpose(2, 0, 1, 3)).astype(bf16)
        xT_in = np.ascontiguousarray(
            xc.reshape(NB, N, ICH, P).transpose(3, 0, 2, 1)).astype(bf16)
        xb = xc.sum(axis=1) / C                          # [NB, DI] fp32
        xbar_in = np.ascontiguousarray(
            xb.reshape(NB, ICH, P).transpose(2, 1, 0)).astype(bf16)
        in_maps.append(
            {"x": x_in, "xT": xT_in, "kk": kk, "kt": kt, "xbar": xbar_in}
        )
    return in_maps


def kernel(x, kernel, _trace=False, _reps=0):
    from concourse.bass_utils import run_bass_kernel_spmd

    x = np.ascontiguousarray(np.asarray(x, dtype=np.float32))
    kernel = np.ascontiguousarray(np.asarray(kernel, dtype=np.float32))
    assert x.shape == (B, N, DI) and kernel.shape == (DI, C * D)

    key = ("nc", _reps)
    if key not in _cache:
        _cache[key] = _build_program(reps=_reps)
    nc = _cache[key]

    in_maps = _prep_inputs(x, kernel)
    res = run_bass_kernel_spmd(nc, in_maps, list(range(NCORES)), trace=_trace)
    _cache["last_result"] = res

    out = np.empty((B, C, D), dtype=np.float32)
    for s in range(NCORES):
        o = res.results[s]["out"]                        # [(c,b), d]
        out[s * NB:(s + 1) * NB] = o.reshape(C, NB, D).transpose(1, 0, 2)
    return out


# revision 25
# speedup vs baseline: 1.3426x; 1.1455x over previous
"""Trainium2 Bass kernel for capsule dynamic routing (nn_Capsule).

Reference computation:
    hat = (x @ kernel).reshape(B, N, C, D).transpose(0, 2, 1, 3)   # [B,C,N,D]
    b = 0; 3 routing iterations of:
        w = softmax(b, axis=capsules)
        o = squash(einsum('bcn,bcnd->bcd', w, hat))
        b += einsum('bcd,bcnd->bcn', o, hat)

Key reformulation (hat is never materialized):
    o[c,d]  = sum_i xw[c,i] * K[i,(c,d)]      xw = w[c,:] @ x      (A-op)
    bupd[c,n] = sum_i x[n,i] * oK[c,i]        oK[c,i] = sum_d o[c,d]*K[i,(c,d)]
This reduces 34 GFLOP of hat-work to ~100 MFLOP of small matmuls whose cost
is streaming K through the PE as stationary operands (bf16, fp32 accumulate).

Sharding: data-parallel over batch B=16 across 8 cores (2 items/core, fused
into the same matmuls via a 2-wide moving operand). kernel K replicated.
"""

import numpy as np
import ml_dtypes
from contextlib import ExitStack

NCORES = 8
B, N, DI = 16, 512, 256         # batch, input capsules, input dim
C, D = 64, 128                  # output capsules, capsule dim
NB = B // NCORES                # batch items per core
P = 128                         # SBUF partitions
NCH = N // P                    # 4 n-chunks
ICH = DI // P                   # 2 i-chunks
ROUTINGS = 3
EPS = 1e-7

_cache = {}


def _build_program(reps=0):
    """reps=0: plain single-shot program (graded path).
    reps>0: wrap the whole body (input DMA + compute + output DMA) in a
    For_i loop for wall-clock-difference benchmarking."""
    import concourse.bass_isa as bass_isa
    import concourse.mybir as mybir
    import concourse.tile as tile
    from concourse import bacc

    F32 = mybir.dt.float32
    BF16 = mybir.dt.bfloat16
    AF = mybir.ActivationFunctionType
    AX = mybir.AxisListType

    class _OneActSetBacc(bacc.Bacc):
        """Every activation func used here (Square/Ln/Exp/Copy) lives in the
        'natural_log_exp_and_others' table set, but the default chooser picks
        per-func sets greedily and flip-flops (one ~1.3us LoadActFuncSet per
        switch, on the critical path). Mask the other sets so exactly one
        table load is emitted; indices are preserved so act_func_set_id still
        points at the real act_info.json entry."""

        def insert_act_table_loads(self):
            import bass_rust as _br
            from concourse.hw_specs import get_activation_tables

            has_activation = any(
                isinstance(i, mybir.InstActivation)
                for b in self.main_func.blocks
                for i in b.instructions
            )
            if not has_activation:
                return
            tables = [
                (name, funcs if name == "natural_log_exp_and_others" else set())
                for name, funcs in get_activation_tables(self.m.arch).items()
            ]
            _br.insert_act_table_loads(self, tables)

    nc = _OneActSetBacc("TRN2", target_bir_lowering=False, debug=False)

    x_d = nc.dram_tensor("x", [P, NB, NCH, DI], BF16, kind="ExternalInput").ap()
    xT_d = nc.dram_tensor("xT", [P, NB, ICH, N], BF16, kind="ExternalInput").ap()
    kk_d = nc.dram_tensor("kk", [P, ICH, C * D], BF16, kind="ExternalInput").ap()
    kt_d = nc.dram_tensor("kt", [P, C, DI], BF16, kind="ExternalInput").ap()
    xbar_d = nc.dram_tensor("xbar", [P, ICH, NB], BF16, kind="ExternalInput").ap()
    # output is [d, (c,b)]; the host does the final transpose to [b, c, d]
    out_d = nc.dram_tensor("out", [P, C * NB], F32, kind="ExternalOutput").ap()

    with tile.TileContext(nc) as tc, ExitStack() as ctx:
        big = ctx.enter_context(tc.tile_pool(name="big", bufs=1))
        wk = ctx.enter_context(tc.tile_pool(name="wk", bufs=2))
        psA = ctx.enter_context(tc.tile_pool(name="psA", bufs=2, space="PSUM"))
        psk = ctx.enter_context(tc.tile_pool(name="psk", bufs=2, space="PSUM"))
        pss = ctx.enter_context(tc.tile_pool(name="pss", bufs=3, space="PSUM"))

        kk = big.tile([P, ICH, C * D], BF16)
        kt = big.tile([P, C, DI], BF16)
        xs = big.tile([P, NB, NCH, DI], BF16)
        xT = big.tile([P, NB, ICH, N], BF16)
        xbar = big.tile([P, ICH, NB], BF16)

        def emit_input_dmas():
            # DMA bandwidth is shared; order by first use: xbar + kk feed the
            # iteration-0 A-op, kt feeds b_op, xT feeds bupd, xs feeds xwT.
            # kk/kt are chunked so capsule matmuls start while later chunks
            # are still in flight.
            nc.sync.dma_start(out=xbar, in_=xbar_d)
            KCH = 8
            kw = (C * D) // KCH
            for j in range(KCH):
                nc.sync.dma_start(
                    out=kk[:, :, j * kw:(j + 1) * kw],
                    in_=kk_d[:, :, j * kw:(j + 1) * kw],
                )
            for j in range(KCH):
                cs = C // KCH
                nc.sync.dma_start(
                    out=kt[:, j * cs:(j + 1) * cs, :],
                    in_=kt_d[:, j * cs:(j + 1) * cs, :],
                )
            nc.sync.dma_start(out=xT, in_=xT_d)
            nc.sync.dma_start(out=xs, in_=x_d)

        bT = big.tile([P, NB, NCH, C], F32)     # routing logits, [n, b, nch, c]
        o_bf = big.tile([P, C, NB], BF16)       # current (unscaled) o, [d, (c,b)]
        epst = big.tile([P, 1], F32)
        nc.vector.memset(epst, EPS)
        zerot = big.tile([P, 1], F32)
        nc.vector.memset(zerot, 0.0)

        def a_op(rhs_at):
            """o_pre[d,(c,b)] = sum_i rhs[i,(c,b)] * K[i,(c,d)] per capsule."""
            po = psA.tile([P, C, NB], F32, tag="po")
            for c in range(C):
                for t in range(ICH):
                    nc.tensor.matmul(
                        po[:, c, :],
                        lhsT=kk[:, t, c * D:(c + 1) * D],
                        rhs=rhs_at(t, c),
                        start=(t == 0),
                        stop=(t == ICH - 1),
                    )
            return po

        def squash_scale(po):
            """scale[c,b] = sqrt(s+eps)/(0.5+s+eps), s = sum_d o_pre[d,(c,b)]^2,
            computed in free layout [*, (c,b)] (identical rows) so it can be
            applied with free-dim broadcasts downstream. The squash scale
            commutes past the (linear) oK matmul, so the PE proceeds straight
            from the A-op into the oK matmuls while this runs on ACT/Pool/DVE.
            Ln/Exp/Square share one activation-table set (no reloads);
            sqrt(t) = exp(0.5*ln(t))."""
            po2 = po[:].rearrange("p c b -> p (c b)")
            sq = wk.tile([P, C * NB], F32, tag="sq")
            nc.scalar.activation(out=sq, in_=po2, func=AF.Square)
            S = wk.tile([P, C * NB], F32, tag="S")
            nc.gpsimd.partition_all_reduce(S, sq, P, bass_isa.ReduceOp.add)
            num = wk.tile([P, C * NB], F32, tag="num")
            nc.scalar.activation(out=num, in_=S, func=AF.Ln, bias=epst[:])
            nc.scalar.activation(out=num, in_=num, func=AF.Exp, scale=0.5,
                                 bias=zerot[:])
            den = wk.tile([P, C * NB], F32, tag="den")
            nc.vector.tensor_scalar_add(den, S, 0.5 + EPS)
            nc.vector.reciprocal(den, den)
            scale = wk.tile([P, C * NB], F32, tag="scalef")
            nc.vector.tensor_mul(scale, num, den)
            return scale

        def b_op(it, scale):
            # oK_pre[i,(c,b)] per i-tile: contraction over d on partitions.
            pk = psk.tile([P, ICH, C, NB], F32, tag="pk")
            for c in range(C):
                for t in range(ICH):
                    nc.tensor.matmul(
                        pk[:, t, c, :],
                        lhsT=kt[:, c, t * P:(t + 1) * P],
                        rhs=o_bf[:, c, :],
                        start=True,
                        stop=True,
                    )
            # apply the squash scale during the PSUM->SBUF eviction
            oks = wk.tile([P, NB, ICH, C], BF16, tag="oks")
            sc3 = scale[:].rearrange("p (c b) -> p b c", b=NB)
            for b in range(NB):
                nc.vector.tensor_tensor(
                    oks[:, b], pk[:, :, :, b],
                    sc3[:, b, None, :].to_broadcast((P, ICH, C)),
                    mybir.AluOpType.mult,
                )
            # bupd[n,(c)] = sum_i x[n,i] oK[c,i]  (lhsT = xT tiles); all four
            # n-tiles of one batch item share a PSUM tile -> one eviction.
            for b in range(NB):
                pb = pss.tile([P, NCH, C], F32, tag="pb")
                for nt in range(NCH):
                    for t in range(ICH):
                        nc.tensor.matmul(
                            pb[:, nt, :],
                            lhsT=xT[:, b, t, nt * P:(nt + 1) * P],
                            rhs=oks[:, b, t, :],
                            start=(t == 0),
                            stop=(t == ICH - 1),
                        )
                if it == 0:
                    nc.vector.tensor_copy(out=bT[:, b], in_=pb)
                else:
                    nc.vector.tensor_add(out=bT[:, b], in0=bT[:, b], in1=pb)

        def softmax_xw():
            # softmax over capsules (innermost free axis of bT); values are
            # O(1) so the max-subtraction is unnecessary.
            e = wk.tile([P, NB, NCH, C], F32, tag="e")
            nc.scalar.activation(out=e, in_=bT[:], func=AF.Exp, bias=zerot[:])
            es = wk.tile([P, NB, NCH], F32, tag="es")
            nc.vector.reduce_sum(out=es, in_=e, axis=AX.X)
            nc.vector.reciprocal(es, es)
            w = wk.tile([P, NB, NCH, C], BF16, tag="w")
            nc.vector.tensor_tensor(
                w, e, es[:, :, :, None].to_broadcast((P, NB, NCH, C)),
                mybir.AluOpType.mult,
            )
            # xwT[i,(c,b)] = sum_n x[n,i] w[n,c]  (lhsT = x tiles); both
            # i-tiles of one batch item share a PSUM tile -> one eviction.
            xwT = wk.tile([P, ICH, C, NB], BF16, tag="xwT")
            for b in range(NB):
                px = pss.tile([P, ICH, C], F32, tag="pb")
                for t in range(ICH):
                    for ch in range(NCH):
                        nc.tensor.matmul(
                            px[:, t, :],
                            lhsT=xs[:, b, ch, t * P:(t + 1) * P],
                            rhs=w[:, b, ch, :],
                            start=(ch == 0),
                            stop=(ch == NCH - 1),
                        )
                nc.vector.tensor_copy(out=xwT[:, :, :, b], in_=px)
            return xwT

        def body():
            emit_input_dmas()
            po = a_op(lambda t, c: xbar[:, t, :])
            for it in range(ROUTINGS - 1):
                nc.vector.tensor_copy(
                    out=o_bf[:].rearrange("p c b -> p (c b)"),
                    in_=po[:].rearrange("p c b -> p (c b)"),
                )
                scale = squash_scale(po)
                b_op(it, scale)
                xwT = softmax_xw()
                po = a_op(lambda t, c, _x=xwT: _x[:, t, c, :])
            # final squash: o = o_pre * scale, emitted as [d, (c,b)]
            scale = squash_scale(po)
            oout = wk.tile([P, C * NB], F32, tag="oout")
            nc.vector.tensor_mul(oout, po[:].rearrange("p c b -> p (c b)"), scale)
            nc.sync.dma_start(out=out_d, in_=oout)

        if reps:
            with tc.For_i(0, reps, 1, hint_engines=(mybir.EngineType.PE,)):
                body()
        else:
            body()

    nc.compile()
    return nc


def _prep_inputs(x, kernel):
    bf16 = ml_dtypes.bfloat16
    kk = np.ascontiguousarray(
        kernel.reshape(ICH, P, C * D).transpose(1, 0, 2)).astype(bf16)
    kt = np.ascontiguousarray(
        kernel.reshape(DI, C, D).transpose(2, 1, 0)).astype(bf16)
    in_maps = []
    for s in range(NCORES):
        xc = x[s * NB:(s + 1) * NB]                      # [NB, N, DI]
        x_in = np.ascontiguousarray(
            xc.reshape(NB, NCH, P, DI).transpose(2, 0, 1, 3)).astype(bf16)
        xT_in = np.ascontiguousarray(
            xc.reshape(NB, N, ICH, P).transpose(3, 0, 2, 1)).astype(bf16)
        xb = xc.sum(axis=1) / C                          # [NB, DI] fp32
        xbar_in = np.ascontiguousarray(
            xb.reshape(NB, ICH, P).transpose(2, 1, 0)).astype(bf16)
        in_maps.append(
            {"x": x_in, "xT": xT_in, "kk": kk, "kt": kt, "xbar": xbar_in}
        )
    return in_maps


def kernel(x, kernel, _trace=False, _reps=0):
    from concourse.bass_utils import run_bass_kernel_spmd

    x = np.ascontiguousarray(np.asarray(x, dtype=np.float32))
    kernel = np.ascontiguousarray(np.asarray(kernel, dtype=np.float32))
    assert x.shape == (B, N, DI) and kernel.shape == (DI, C * D)

    key = ("nc", _reps)
    if key not in _cache:
        _cache[key] = _build_program(reps=_reps)
    nc = _cache[key]

    in_maps = _prep_inputs(x, kernel)
    res = run_bass_kernel_spmd(nc, in_maps, list(range(NCORES)), trace=_trace)
    _cache["last_result"] = res

    out = np.empty((B, C, D), dtype=np.float32)
    for s in range(NCORES):
        o = res.results[s]["out"]                        # [d, (c,b)]
        out[s * NB:(s + 1) * NB] = o.reshape(D, C, NB).transpose(2, 1, 0)
    return out


# revision 26
# speedup vs baseline: 1.4388x; 1.0716x over previous
"""Trainium2 Bass kernel for capsule dynamic routing (nn_Capsule).

Reference computation:
    hat = (x @ kernel).reshape(B, N, C, D).transpose(0, 2, 1, 3)   # [B,C,N,D]
    b = 0; 3 routing iterations of:
        w = softmax(b, axis=capsules)
        o = squash(einsum('bcn,bcnd->bcd', w, hat))
        b += einsum('bcd,bcnd->bcn', o, hat)

Key reformulation (hat is never materialized):
    o[c,d]  = sum_i xw[c,i] * K[i,(c,d)]      xw = w[c,:] @ x      (A-op)
    bupd[c,n] = sum_i x[n,i] * oK[c,i]        oK[c,i] = sum_d o[c,d]*K[i,(c,d)]
This reduces 34 GFLOP of hat-work to ~100 MFLOP of small matmuls whose cost
is streaming K through the PE as stationary operands (bf16, fp32 accumulate).

Sharding: data-parallel over batch B=16 across 8 cores (2 items/core, fused
into the same matmuls via a 2-wide moving operand). kernel K replicated.
"""

import numpy as np
import ml_dtypes
from contextlib import ExitStack

NCORES = 8
B, N, DI = 16, 512, 256         # batch, input capsules, input dim
C, D = 64, 128                  # output capsules, capsule dim
NB = B // NCORES                # batch items per core
P = 128                         # SBUF partitions
NCH = N // P                    # 4 n-chunks
ICH = DI // P                   # 2 i-chunks
ROUTINGS = 3
EPS = 1e-7

_cache = {}


def _build_program(reps=0):
    """reps=0: plain single-shot program (graded path).
    reps>0: wrap the whole body (input DMA + compute + output DMA) in a
    For_i loop for wall-clock-difference benchmarking."""
    import concourse.bass_isa as bass_isa
    import concourse.mybir as mybir
    import concourse.tile as tile
    from concourse import bacc

    F32 = mybir.dt.float32
    BF16 = mybir.dt.bfloat16
    AF = mybir.ActivationFunctionType
    AX = mybir.AxisListType

    class _OneActSetBacc(bacc.Bacc):
        """Every activation func used here (Square/Ln/Exp/Copy) lives in the
        'natural_log_exp_and_others' table set, but the default chooser picks
        per-func sets greedily and flip-flops (one ~1.3us LoadActFuncSet per
        switch, on the critical path). Mask the other sets so exactly one
        table load is emitted; indices are preserved so act_func_set_id still
        points at the real act_info.json entry."""

        def insert_act_table_loads(self):
            import bass_rust as _br
            from concourse.hw_specs import get_activation_tables

            has_activation = any(
                isinstance(i, mybir.InstActivation)
                for b in self.main_func.blocks
                for i in b.instructions
            )
            if not has_activation:
                return
            tables = [
                (name, funcs if name == "natural_log_exp_and_others" else set())
                for name, funcs in get_activation_tables(self.m.arch).items()
            ]
            _br.insert_act_table_loads(self, tables)

    nc = _OneActSetBacc("TRN2", target_bir_lowering=False, debug=False)

    x_d = nc.dram_tensor("x", [P, NB, NCH, DI], BF16, kind="ExternalInput").ap()
    xT_d = nc.dram_tensor("xT", [P, NB, ICH, N], BF16, kind="ExternalInput").ap()
    kk_d = nc.dram_tensor("kk", [P, ICH, C * D], BF16, kind="ExternalInput").ap()
    kt_d = nc.dram_tensor("kt", [P, C, DI], BF16, kind="ExternalInput").ap()
    xbar_d = nc.dram_tensor("xbar", [P, ICH, NB], BF16, kind="ExternalInput").ap()
    # output is [d, (c,b)]; the host does the final transpose to [b, c, d]
    out_d = nc.dram_tensor("out", [P, C * NB], F32, kind="ExternalOutput").ap()

    with tile.TileContext(nc) as tc, ExitStack() as ctx:
        big = ctx.enter_context(tc.tile_pool(name="big", bufs=1))
        wk = ctx.enter_context(tc.tile_pool(name="wk", bufs=2))
        psA = ctx.enter_context(tc.tile_pool(name="psA", bufs=2, space="PSUM"))
        psk = ctx.enter_context(tc.tile_pool(name="psk", bufs=2, space="PSUM"))
        pss = ctx.enter_context(tc.tile_pool(name="pss", bufs=3, space="PSUM"))

        kk = big.tile([P, ICH, C * D], BF16)
        kt = big.tile([P, C, DI], BF16)
        xs = big.tile([P, NB, NCH, DI], BF16)
        xT = big.tile([P, NB, ICH, N], BF16)
        xbar = big.tile([P, ICH, NB], BF16)

        def emit_input_dmas():
            # DMA bandwidth is shared; order by first use: xbar + kk feed the
            # iteration-0 A-op, kt feeds b_op, xT feeds bupd, xs feeds xwT.
            # kk/kt are chunked so capsule matmuls start while later chunks
            # are still in flight.
            nc.sync.dma_start(out=xbar, in_=xbar_d)
            KCH = 8
            kw = (C * D) // KCH
            for j in range(KCH):
                nc.sync.dma_start(
                    out=kk[:, :, j * kw:(j + 1) * kw],
                    in_=kk_d[:, :, j * kw:(j + 1) * kw],
                )
            for j in range(KCH):
                cs = C // KCH
                nc.sync.dma_start(
                    out=kt[:, j * cs:(j + 1) * cs, :],
                    in_=kt_d[:, j * cs:(j + 1) * cs, :],
                )
            nc.sync.dma_start(out=xT, in_=xT_d)
            nc.sync.dma_start(out=xs, in_=x_d)

        bT = big.tile([P, NB, NCH, C], F32)     # routing logits, [n, b, nch, c]
        o_bf = big.tile([P, C, NB], BF16)       # current (unscaled) o, [d, (c,b)]
        epst = big.tile([P, 1], F32)
        nc.vector.memset(epst, EPS)
        zerot = big.tile([P, 1], F32)
        nc.vector.memset(zerot, 0.0)
        # Dummy activation up front so the one LoadActFuncSet (~1.3us) runs
        # during the initial DMA wait instead of on the critical path.
        warm = big.tile([P, 1], F32)
        nc.scalar.activation(out=warm, in_=zerot[:], func=AF.Exp, bias=zerot[:])

        def a_op(rhs_at):
            """o_pre[d,(c,b)] = sum_i rhs[i,(c,b)] * K[i,(c,d)] per capsule."""
            po = psA.tile([P, C, NB], F32, tag="po")
            for c in range(C):
                for t in range(ICH):
                    nc.tensor.matmul(
                        po[:, c, :],
                        lhsT=kk[:, t, c * D:(c + 1) * D],
                        rhs=rhs_at(t, c),
                        start=(t == 0),
                        stop=(t == ICH - 1),
                    )
            return po

        def squash_scale(po):
            """scale[c,b] = sqrt(s+eps)/(0.5+s+eps), s = sum_d o_pre[d,(c,b)]^2,
            computed in free layout [*, (c,b)] (identical rows) so it can be
            applied with free-dim broadcasts downstream. The squash scale
            commutes past the (linear) oK matmul, so the PE proceeds straight
            from the A-op into the oK matmuls while this runs on ACT/Pool/DVE.
            Ln/Exp/Square share one activation-table set (no reloads);
            sqrt(t) = exp(0.5*ln(t))."""
            po2 = po[:].rearrange("p c b -> p (c b)")
            sq = wk.tile([P, C * NB], F32, tag="sq")
            nc.scalar.activation(out=sq, in_=po2, func=AF.Square)
            S = wk.tile([P, C * NB], F32, tag="S")
            nc.gpsimd.partition_all_reduce(S, sq, P, bass_isa.ReduceOp.add)
            num = wk.tile([P, C * NB], F32, tag="num")
            nc.scalar.activation(out=num, in_=S, func=AF.Ln, bias=epst[:])
            nc.scalar.activation(out=num, in_=num, func=AF.Exp, scale=0.5,
                                 bias=zerot[:])
            den = wk.tile([P, C * NB], F32, tag="den")
            nc.vector.tensor_scalar_add(den, S, 0.5 + EPS)
            nc.vector.reciprocal(den, den)
            scale = wk.tile([P, C * NB], F32, tag="scalef")
            nc.vector.tensor_mul(scale, num, den)
            return scale

        def b_op(it, scale):
            # oK_pre[i,(c,b)] per i-tile: contraction over d on partitions.
            pk = psk.tile([P, ICH, C, NB], F32, tag="pk")
            for c in range(C):
                for t in range(ICH):
                    nc.tensor.matmul(
                        pk[:, t, c, :],
                        lhsT=kt[:, c, t * P:(t + 1) * P],
                        rhs=o_bf[:, c, :],
                        start=True,
                        stop=True,
                    )
            # apply the squash scale during the PSUM->SBUF eviction
            oks = wk.tile([P, NB, ICH, C], BF16, tag="oks")
            sc3 = scale[:].rearrange("p (c b) -> p b c", b=NB)
            for b in range(NB):
                nc.vector.tensor_tensor(
                    oks[:, b], pk[:, :, :, b],
                    sc3[:, b, None, :].to_broadcast((P, ICH, C)),
                    mybir.AluOpType.mult,
                )
            # bupd[n,(c)] = sum_i x[n,i] oK[c,i]  (lhsT = xT tiles); all four
            # n-tiles of one batch item share a PSUM tile -> one eviction.
            for b in range(NB):
                pb = pss.tile([P, NCH, C], F32, tag="pb")
                for nt in range(NCH):
                    for t in range(ICH):
                        nc.tensor.matmul(
                            pb[:, nt, :],
                            lhsT=xT[:, b, t, nt * P:(nt + 1) * P],
                            rhs=oks[:, b, t, :],
                            start=(t == 0),
                            stop=(t == ICH - 1),
                        )
                if it == 0:
                    nc.vector.tensor_copy(out=bT[:, b], in_=pb)
                else:
                    nc.vector.tensor_add(out=bT[:, b], in0=bT[:, b], in1=pb)

        def softmax_xw():
            # softmax over capsules (innermost free axis of bT); values are
            # O(1) so the max-subtraction is unnecessary.
            e = wk.tile([P, NB, NCH, C], F32, tag="e")
            nc.scalar.activation(out=e, in_=bT[:], func=AF.Exp, bias=zerot[:])
            es = wk.tile([P, NB, NCH], F32, tag="es")
            nc.vector.reduce_sum(out=es, in_=e, axis=AX.X)
            nc.vector.reciprocal(es, es)
            w = wk.tile([P, NB, NCH, C], BF16, tag="w")
            nc.vector.tensor_tensor(
                w, e, es[:, :, :, None].to_broadcast((P, NB, NCH, C)),
                mybir.AluOpType.mult,
            )
            # xwT[i,(c,b)] = sum_n x[n,i] w[n,c]  (lhsT = x tiles); both
            # i-tiles of one batch item share a PSUM tile -> one eviction.
            xwT = wk.tile([P, ICH, C, NB], BF16, tag="xwT")
            for b in range(NB):
                px = pss.tile([P, ICH, C], F32, tag="pb")
                for t in range(ICH):
                    for ch in range(NCH):
                        nc.tensor.matmul(
                            px[:, t, :],
                            lhsT=xs[:, b, ch, t * P:(t + 1) * P],
                            rhs=w[:, b, ch, :],
                            start=(ch == 0),
                            stop=(ch == NCH - 1),
                        )
                nc.vector.tensor_copy(out=xwT[:, :, :, b], in_=px)
            return xwT

        def body():
            emit_input_dmas()
            po = a_op(lambda t, c: xbar[:, t, :])
            for it in range(ROUTINGS - 1):
                nc.vector.tensor_copy(
                    out=o_bf[:].rearrange("p c b -> p (c b)"),
                    in_=po[:].rearrange("p c b -> p (c b)"),
                )
                scale = squash_scale(po)
                b_op(it, scale)
                xwT = softmax_xw()
                po = a_op(lambda t, c, _x=xwT: _x[:, t, c, :])
            # final squash: o = o_pre * scale, emitted as [d, (c,b)]
            scale = squash_scale(po)
            oout = wk.tile([P, C * NB], F32, tag="oout")
            nc.vector.tensor_mul(oout, po[:].rearrange("p c b -> p (c b)"), scale)
            nc.sync.dma_start(out=out_d, in_=oout)

        if reps:
            with tc.For_i(0, reps, 1, hint_engines=(mybir.EngineType.PE,)):
                body()
        else:
            body()

    nc.compile()
    return nc


def _prep_inputs(x, kernel):
    bf16 = ml_dtypes.bfloat16
    kk = np.ascontiguousarray(
        kernel.reshape(ICH, P, C * D).transpose(1, 0, 2)).astype(bf16)
    kt = np.ascontiguousarray(
        kernel.reshape(DI, C, D).transpose(2, 1, 0)).astype(bf16)
    in_maps = []
    for s in range(NCORES):
        xc = x[s * NB:(s + 1) * NB]                      # [NB, N, DI]
        x_in = np.ascontiguousarray(
            xc.reshape(NB, NCH, P, DI).transpose(2, 0, 1, 3)).astype(bf16)
        xT_in = np.ascontiguousarray(
            xc.reshape(NB, N, ICH, P).transpose(3, 0, 2, 1)).astype(bf16)
        xb = xc.sum(axis=1) / C                          # [NB, DI] fp32
        xbar_in = np.ascontiguousarray(
            xb.reshape(NB, ICH, P).transpose(2, 1, 0)).astype(bf16)
        in_maps.append(
            {"x": x_in, "xT": xT_in, "kk": kk, "kt": kt, "xbar": xbar_in}
        )
    return in_maps


def kernel(x, kernel, _trace=False, _reps=0):
    from concourse.bass_utils import run_bass_kernel_spmd

    x = np.ascontiguousarray(np.asarray(x, dtype=np.float32))
    kernel = np.ascontiguousarray(np.asarray(kernel, dtype=np.float32))
    assert x.shape == (B, N, DI) and kernel.shape == (DI, C * D)

    key = ("nc", _reps)
    if key not in _cache:
        _cache[key] = _build_program(reps=_reps)
    nc = _cache[key]

    in_maps = _prep_inputs(x, kernel)
    res = run_bass_kernel_spmd(nc, in_maps, list(range(NCORES)), trace=_trace)
    _cache["last_result"] = res

    out = np.empty((B, C, D), dtype=np.float32)
    for s in range(NCORES):
        o = res.results[s]["out"]                        # [d, (c,b)]
        out[s * NB:(s + 1) * NB] = o.reshape(D, C, NB).transpose(2, 1, 0)
    return out
